# revision 1
# baseline (speedup 1.0000x reference)
"""Trainium2 Bass kernel for the Deepeucloss loss function.

Computes (see math below) a scalar loss from five [16, 128, 4096, 3] f32
tensors plus three scalars.  Data-parallel across 8 NeuronCores: each core
takes 2 of the 16 batches, streams its 60 MiB of inputs through SBUF once,
and emits tiny per-(batch,point) partial sums.  The host combines the 8
partial-stat blocks (an all-reduce of scalars) in float64.

Math (NUM_CLASSES=128, L2_LAMBDA=0.01, S2=2.0):
  euc(m)   = sum_{b,p} sqrt(sum_{n,d} (m - target)^2) / 128
  base     = log(2/s1) + s1^2/8 - 0.5          (s1 = gt2_var)
  kl       = 1.4*sum(base) + (S0 + 0.2*S1 + 0.2*S2)/8,
             Sk = sum((m_k - target)^2)
  outloss  = euc(out) + 0.002*l_dynamic*leg
  gt_loss  = 0.1*euc(gt1_mean) + 0.2*euc(gt2_mean)
  reg      = gt0 * 0.01 * l_dynamic
  result   = outloss + gt_loss + reg + kl / (1.2*(euc(out) + gt_loss))

Device kernel per core: for each [128, CHUNK] tile, DVE computes the three
differences and ACT does the five fused square/ln free-axis accumulations
(one accumulator column per chunk).  Output: [5, 128, 12] partial sums.
Tuning (measured via interleaved repetition-delta): CHUNK=2048 beats 1024
by ~22 us/pass (per-DMA overhead); io bufs=3 beats bufs=2 by ~13 us/pass
(keeps more loads in flight across compute jitter).  ~158 us/pass vs the
~175 us DMA roofline estimate @360 GB/s.
"""

from contextlib import nullcontext

import numpy as np

import concourse.bacc as bacc
import concourse.tile as tile
import concourse.mybir as mybir
from concourse import bass_utils

B, P, N, D = 16, 128, 4096, 3
F = N * D                      # 12288 elements per (batch, point) row
NCORES = 8
BL = B // NCORES               # batches per core
CHUNK = 2048
NCHUNK = F // CHUNK            # chunks per row
NACC = BL * NCHUNK             # accumulator columns per core
CORE_IDS = list(range(NCORES))

IN_NAMES = ("t_out", "t_tgt", "t_gt1", "t_gt2", "t_s1")

_CACHE = {}
LAST_RESULTS = None            # BassKernelResults of the most recent run


def _build(reps=1):
    # reps>1 wraps the streaming loop in a hardware For_i (same result; every
    # repetition recomputes the same stats) — used only for repetition-delta
    # timing in test.py.  The graded path always builds with reps=1.
    fp32 = mybir.dt.float32
    nc = bacc.Bacc(
        "TRN2", target_bir_lowering=False, debug=False, num_devices=NCORES
    )
    ins = {
        name: nc.dram_tensor(name, [BL, P, F], fp32, kind="ExternalInput").ap()
        for name in IN_NAMES
    }
    stats = nc.dram_tensor("stats", [5, P, NACC], fp32, kind="ExternalOutput").ap()

    Sq = mybir.ActivationFunctionType.Square
    Ln = mybir.ActivationFunctionType.Ln

    with tile.TileContext(nc) as tc:
        with (
            tc.tile_pool(name="io", bufs=3) as io_pool,
            tc.tile_pool(name="dif", bufs=2) as dif_pool,
            tc.tile_pool(name="scr", bufs=1) as scr_pool,
            tc.tile_pool(name="acc", bufs=1) as acc_pool,
        ):
            accs = [
                acc_pool.tile([P, NACC], fp32, tag=f"acc{k}", name=f"acc{k}")
                for k in range(5)
            ]
            scr_act = scr_pool.tile([P, CHUNK], fp32, tag="scr_act", name="scr_act")

            rep_loop = tc.For_i(0, reps, 1) if reps > 1 else nullcontext()
            with rep_loop:
                for t in range(BL):
                    for c in range(NCHUNK):
                        idx = t * NCHUNK + c
                        cs = slice(c * CHUNK, (c + 1) * CHUNK)
                        tl = {}
                        for name in IN_NAMES:
                            tl[name] = io_pool.tile(
                                [P, CHUNK], fp32, tag=name, name=name
                            )
                            nc.sync.dma_start(tl[name][:], ins[name][t, :, cs])

                        d0 = dif_pool.tile([P, CHUNK], fp32, tag="d0", name="d0")
                        nc.vector.tensor_sub(d0[:], tl["t_out"][:], tl["t_tgt"][:])
                        d1 = dif_pool.tile([P, CHUNK], fp32, tag="d1", name="d1")
                        nc.vector.tensor_sub(d1[:], tl["t_gt1"][:], tl["t_tgt"][:])
                        d2 = dif_pool.tile([P, CHUNK], fp32, tag="d2", name="d2")
                        nc.vector.tensor_sub(d2[:], tl["t_gt2"][:], tl["t_tgt"][:])

                        for k, d in enumerate((d0, d1, d2)):
                            nc.scalar.activation(
                                scr_act[:], d[:], Sq,
                                accum_out=accs[k][:, idx : idx + 1],
                            )
                        nc.scalar.activation(
                            scr_act[:], tl["t_s1"][:], Ln,
                            accum_out=accs[3][:, idx : idx + 1],
                        )
                        # tensor_tensor_reduce (DVE) crashes the PJRT/axon
                        # HW path, so s1^2 goes through ACT like the others.
                        nc.scalar.activation(
                            scr_act[:], tl["t_s1"][:], Sq,
                            accum_out=accs[4][:, idx : idx + 1],
                        )

            for k in range(5):
                nc.sync.dma_start(stats[k], accs[k][:])

    nc.compile()
    return nc


def _get_nc():
    if "nc" not in _CACHE:
        _CACHE["nc"] = _build()
    return _CACHE["nc"]


def kernel(out, target, gt0, gt1_mean, gt2_mean, gt2_var, leg, l_dynamic):
    global LAST_RESULTS
    nc = _get_nc()

    def shard(arr):
        arr = np.ascontiguousarray(np.asarray(arr, dtype=np.float32))
        return [arr[i * BL : (i + 1) * BL].reshape(BL, P, F) for i in CORE_IDS]

    shards = {
        "t_out": shard(out),
        "t_tgt": shard(target),
        "t_gt1": shard(gt1_mean),
        "t_gt2": shard(gt2_mean),
        "t_s1": shard(gt2_var),
    }
    in_maps = [{name: shards[name][i] for name in IN_NAMES} for i in CORE_IDS]

    res = bass_utils.run_bass_kernel_spmd(nc, in_maps, CORE_IDS)
    LAST_RESULTS = res

    # [8, 5, P, NACC] partial sums; reduce chunk columns per (batch, point) row.
    stats = np.stack(
        [np.asarray(r["stats"], dtype=np.float64) for r in res.results]
    )
    rs = stats.reshape(NCORES, 5, P, BL, NCHUNK).sum(axis=4)  # [8, 5, P, BL]

    euc0 = np.sqrt(rs[:, 0]).sum() / 128.0
    euc1 = np.sqrt(rs[:, 1]).sum() / 128.0
    euc2 = np.sqrt(rs[:, 2]).sum() / 128.0
    s0, s1, s2 = rs[:, 0].sum(), rs[:, 1].sum(), rs[:, 2].sum()
    ln_sum, sq_sum = rs[:, 3].sum(), rs[:, 4].sum()

    ntot = float(B * P * N * D)
    base_sum = ntot * np.log(2.0) - ln_sum + sq_sum / 8.0 - 0.5 * ntot
    kl = 1.4 * base_sum + (s0 + 0.2 * s1 + 0.2 * s2) / 8.0

    l_dyn, leg_v, gt0_v = float(l_dynamic), float(leg), float(gt0)
    outloss = euc0 + 0.01 * 0.2 * l_dyn * leg_v
    gt_loss = 0.1 * euc1 + 0.2 * euc2
    reg = gt0_v * 0.01 * l_dyn
    result = outloss + gt_loss + reg + kl / (1.2 * (euc0 + gt_loss))
    return np.asarray(result, dtype=np.float32)



# revision 2
# speedup vs baseline: 2.1105x; 2.1105x over previous
"""Trainium2 Bass kernel for the Deepeucloss loss function.

Computes a scalar loss from five [16, 128, 4096, 3] f32 tensors plus three
scalars.  Data-parallel across 8 NeuronCores: each core takes 2 of the 16
batches and streams its five input shards through SBUF once.

Math (NUM_CLASSES=128, L2_LAMBDA=0.01, S2=2.0):
  euc(m)   = sum_{b,p} sqrt(sum_{n,d} (m - target)^2) / 128
  base     = log(2/s1) + s1^2/8 - 0.5          (s1 = gt2_var)
  kl       = 1.4*sum(base) + (S0 + 0.2*S1 + 0.2*S2)/8,
             Sk = sum((m_k - target)^2)
  outloss  = euc(out) + 0.002*l_dynamic*leg
  gt_loss  = 0.1*euc(gt1_mean) + 0.2*euc(gt2_mean)
  reg      = gt0 * 0.01 * l_dynamic
  result   = outloss + gt_loss + reg + kl / (1.2*(euc(out) + gt_loss))

v2 (this file): inputs are converted to bf16 on the host, halving HBM
traffic — the f32 baseline sat at the ~358 GB/s per-core HBM roofline
(176 us), so bf16 moves the DMA floor to ~88 us.  At bf16 the five ACT
accumulation passes (1 elem/lane/cycle regardless of dtype, ~107 us) would
become the bottleneck, so the per-tile square-sum reductions are split
between ACT (Square/Ln activation with accum_out) and DVE (bn_stats, which
fuses square+reduce: sum(x^2) = count*var + count*mean^2).  The Ln pass
stays on ACT (only engine with transcendentals); DVE additionally computes
the three bf16 diffs at 2x rate.  Per-core engine-busy estimates:
DMA ~88 us, ACT ~80 us, DVE ~80 us.

Device output per core: "acc" [128, n_act_jobs] f32 activation accum
columns and "bn" [128, n_dve_jobs, CHUNK/512, 6] f32 bn_stats blocks.  The
host reconstructs the five per-(batch,point) / global sums in float64 and
finishes the scalar algebra.
"""

from contextlib import nullcontext

import numpy as np
import ml_dtypes

import concourse.bacc as bacc
import concourse.tile as tile
import concourse.mybir as mybir
from concourse import bass_utils

B, P, N, D = 16, 128, 4096, 3
F = N * D                      # 12288 elements per (batch, point) row
NCORES = 8
BL = B // NCORES               # batches per core
CHUNK = 4096
NCHUNK = F // CHUNK            # chunks per row
BNSEG = CHUNK // 512           # bn_stats segments per chunk
CORE_IDS = list(range(NCORES))

IN_NAMES = ("t_out", "t_tgt", "t_gt1", "t_gt2", "t_s1")

# Stat jobs: for each tile (t, c) there are 4 square-sum stats
# (d0^2, d1^2, d2^2, s1^2) plus an Ln(s1) stat that is always on ACT.
# Each square-sum job is assigned to ACT (Square + accum_out) or DVE
# (bn_stats) by the schedule below; the host reads the same table.
# STAT keys: 0=d0, 1=d1, 2=d2, 3=s1sq
N_TILES = BL * NCHUNK          # 6 tiles of [128, CHUNK] per pass
ALL_JOBS = [(k, t, c) for t in range(BL) for c in range(NCHUNK)
            for k in range(4)]
# 24 square jobs; put N_DVE_JOBS of them on DVE (bn_stats), rest on ACT.
N_DVE_JOBS = 9
# round-robin over position so both engines' work interleaves smoothly
DVE_JOBS = [j for i, j in enumerate(ALL_JOBS) if i % 8 in (0, 3, 5)]
ACT_JOBS = [j for j in ALL_JOBS if j not in DVE_JOBS]
assert len(DVE_JOBS) == N_DVE_JOBS

_CACHE = {}
LAST_RESULTS = None


def _build(reps=1):
    # reps>1 wraps the streaming loop in a hardware For_i (same result; every
    # repetition recomputes the same stats) — used only for repetition-delta
    # timing in test.py.  The graded path always builds with reps=1.
    fp32 = mybir.dt.float32
    bf16 = mybir.dt.bfloat16
    nc = bacc.Bacc(
        "TRN2", target_bir_lowering=False, debug=False, num_devices=NCORES
    )
    ins = {
        name: nc.dram_tensor(name, [BL, P, F], bf16, kind="ExternalInput").ap()
        for name in IN_NAMES
    }
    n_act = len(ACT_JOBS) + N_TILES          # + Ln jobs
    acc_out = nc.dram_tensor("acc", [P, n_act], fp32, kind="ExternalOutput").ap()
    bn_out = nc.dram_tensor(
        "bn", [P, N_DVE_JOBS, BNSEG, 6], fp32, kind="ExternalOutput"
    ).ap()

    Sq = mybir.ActivationFunctionType.Square
    Ln = mybir.ActivationFunctionType.Ln

    act_col = {}                 # (kind, t, c) -> acc column; kind 4 = ln
    for i, j in enumerate(ACT_JOBS):
        act_col[j] = i
    for t in range(BL):
        for c in range(NCHUNK):
            act_col[(4, t, c)] = len(ACT_JOBS) + t * NCHUNK + c
    bn_slot = {j: i for i, j in enumerate(DVE_JOBS)}

    with tile.TileContext(nc) as tc:
        with (
            tc.tile_pool(name="io", bufs=3) as io_pool,
            tc.tile_pool(name="dif", bufs=2) as dif_pool,
            tc.tile_pool(name="scr", bufs=1) as scr_pool,
            tc.tile_pool(name="acc", bufs=1) as acc_pool,
        ):
            acc = acc_pool.tile([P, n_act], fp32, tag="acc", name="acc")
            bn = acc_pool.tile(
                [P, N_DVE_JOBS, BNSEG, 6], fp32, tag="bn", name="bn"
            )
            scr_act = scr_pool.tile([P, CHUNK], bf16, tag="scr_act",
                                    name="scr_act")

            rep_loop = tc.For_i(0, reps, 1) if reps > 1 else nullcontext()
            with rep_loop:
                for t in range(BL):
                    for c in range(NCHUNK):
                        cs = slice(c * CHUNK, (c + 1) * CHUNK)
                        tl = {}
                        for name in IN_NAMES:
                            tl[name] = io_pool.tile(
                                [P, CHUNK], bf16, tag=name, name=name
                            )
                            nc.sync.dma_start(tl[name][:], ins[name][t, :, cs])

                        difs = []
                        for k, name in enumerate(("t_out", "t_gt1", "t_gt2")):
                            d = dif_pool.tile([P, CHUNK], bf16, tag=f"d{k}",
                                              name=f"d{k}")
                            nc.vector.tensor_sub(d[:], tl[name][:],
                                                 tl["t_tgt"][:])
                            difs.append(d)

                        srcs = difs + [tl["t_s1"]]
                        for k in range(4):
                            j = (k, t, c)
                            if j in bn_slot:
                                s = bn_slot[j]
                                src = srcs[k][:].rearrange(
                                    "p (g f) -> p g f", g=BNSEG
                                )
                                for g in range(BNSEG):
                                    nc.vector.bn_stats(
                                        bn[:, s, g, :], src[:, g, :]
                                    )
                            else:
                                nc.scalar.activation(
                                    scr_act[:], srcs[k][:], Sq,
                                    accum_out=acc[:, act_col[j]: act_col[j] + 1],
                                )
                        lcol = act_col[(4, t, c)]
                        nc.scalar.activation(
                            scr_act[:], tl["t_s1"][:], Ln,
                            accum_out=acc[:, lcol: lcol + 1],
                        )

            nc.sync.dma_start(acc_out, acc[:])
            nc.sync.dma_start(bn_out, bn[:])

    nc.compile()
    return nc


def _get_nc():
    if "nc" not in _CACHE:
        _CACHE["nc"] = _build()
    return _CACHE["nc"]


def _to_bf16_shards(arr):
    a = np.asarray(arr, dtype=np.float32).astype(ml_dtypes.bfloat16)
    return [np.ascontiguousarray(a[i * BL:(i + 1) * BL].reshape(BL, P, F))
            for i in CORE_IDS]


def _reduce_stats(results):
    """Reconstruct S0,S1,S2 per (b,p), plus global ln_sum and sq_sum."""
    acc = np.stack([np.asarray(r["acc"], dtype=np.float64)
                    for r in results])          # [8, P, n_act]
    bn = np.stack([np.asarray(r["bn"], dtype=np.float64)
                   for r in results])           # [8, P, n_dve, BNSEG, 6]

    S = np.zeros((4, NCORES, BL, P))            # stat k, core, batch, point
    ln_sum = 0.0
    for i, (k, t, c) in enumerate(ACT_JOBS):
        S[k, :, t, :] += acc[:, :, i]
    for t in range(BL):
        for c in range(NCHUNK):
            ln_sum += acc[:, :, len(ACT_JOBS) + t * NCHUNK + c].sum()
    for s, (k, t, c) in enumerate(DVE_JOBS):
        blk = bn[:, :, s, :, :]                 # [8, P, BNSEG, 6]
        sq = (blk[..., 2] + blk[..., 0] * blk[..., 1] ** 2
              + blk[..., 5] + blk[..., 3] * blk[..., 4] ** 2)
        S[k, :, t, :] += sq.sum(axis=-1)
    return S, ln_sum


def kernel(out, target, gt0, gt1_mean, gt2_mean, gt2_var, leg, l_dynamic):
    global LAST_RESULTS
    nc = _get_nc()

    shards = {
        "t_out": _to_bf16_shards(out),
        "t_tgt": _to_bf16_shards(target),
        "t_gt1": _to_bf16_shards(gt1_mean),
        "t_gt2": _to_bf16_shards(gt2_mean),
        "t_s1": _to_bf16_shards(gt2_var),
    }
    in_maps = [{name: shards[name][i] for name in IN_NAMES} for i in CORE_IDS]

    res = bass_utils.run_bass_kernel_spmd(nc, in_maps, CORE_IDS)
    LAST_RESULTS = res

    S, ln_sum = _reduce_stats(res.results)

    euc0 = np.sqrt(S[0]).sum() / 128.0
    euc1 = np.sqrt(S[1]).sum() / 128.0
    euc2 = np.sqrt(S[2]).sum() / 128.0
    s0, s1, s2 = S[0].sum(), S[1].sum(), S[2].sum()
    sq_sum = S[3].sum()

    ntot = float(B * P * N * D)
    base_sum = ntot * np.log(2.0) - ln_sum + sq_sum / 8.0 - 0.5 * ntot
    kl = 1.4 * base_sum + (s0 + 0.2 * s1 + 0.2 * s2) / 8.0

    l_dyn, leg_v, gt0_v = float(l_dynamic), float(leg), float(gt0)
    outloss = euc0 + 0.01 * 0.2 * l_dyn * leg_v
    gt_loss = 0.1 * euc1 + 0.2 * euc2
    reg = gt0_v * 0.01 * l_dyn
    result = outloss + gt_loss + reg + kl / (1.2 * (euc0 + gt_loss))
    return np.asarray(result, dtype=np.float32)


# revision 12
# speedup vs baseline: 3.4636x; 1.6412x over previous
"""Trainium2 Bass kernel for the Deepeucloss loss function.

Computes a scalar loss from five [16, 128, 4096, 3] f32 tensors plus three
scalars.  Data-parallel across 8 NeuronCores: each core takes 2 of the 16
batches and streams its five input shards through SBUF once.

Math (NUM_CLASSES=128, L2_LAMBDA=0.01, S2=2.0):
  euc(m)   = sum_{b,p} sqrt(sum_{n,d} (m - target)^2) / 128
  base     = log(2/s1) + s1^2/8 - 0.5          (s1 = gt2_var)
  kl       = 1.4*sum(base) + (S0 + 0.2*S1 + 0.2*S2)/8,
             Sk = sum((m_k - target)^2)
  outloss  = euc(out) + 0.002*l_dynamic*leg
  gt_loss  = 0.1*euc(gt1_mean) + 0.2*euc(gt2_mean)
  reg      = gt0 * 0.01 * l_dynamic
  result   = outloss + gt_loss + reg + kl / (1.2*(euc(out) + gt_loss))

v2.6 design.  The f32 baseline sat at the ~358 GB/s per-core HBM roofline
(176 us).  Here the four N(0,1) tensors (out / negated-target / gt1 / gt2)
are stored fp8-e4m3 in HBM (quantization bias ~0.1%, far inside the 2e-2
gate) and gt2_var stays bf16, cutting per-core HBM bytes to 18.9 MB
(~53 us floor).  fp8 is slow on DVE (no 8-bit packing), so the diffs are
computed on the otherwise-idle PE tensor engine:

  d_k = I @ m_k + I @ (-target)     (PSUM fp32 accumulation, 512-col
                                     segments; target is negated on the
                                     host so the identity stationary
                                     never changes -> no weight reloads)

ACT (Square+accum_out) and DVE (bn_stats) then square-and-reduce the
per-(batch,point) sums directly from PSUM, split so both engines stay
under the DMA floor.  Ln(s1) and s1^2 run on ACT from the bf16 s1 tile
(Square is a filler function in the Ln table set -> no table thrash).

Device output per core: "acc" [128, n_cols] f32 activation accum columns
and "bn" [128, n_dve, chunk/512, 6] f32 bn_stats blocks
(sum(x^2) = count*var + count*mean^2).  The host reconstructs all sums in
float64 and finishes the scalar algebra.
"""

from contextlib import nullcontext

import numpy as np
import ml_dtypes

import concourse.bacc as bacc
import concourse.tile as tile
import concourse.mybir as mybir
from concourse import bass_utils

B, P, N, D = 16, 128, 4096, 3
F = N * D                      # 12288 elements per (batch, point) row
NCORES = 8
BL = B // NCORES               # batches per core
CHUNK = 4096
SEG = 512                      # PSUM bank free-dim capacity (fp32)
N_DVE_JOBS = 15
IO_BUFS = 3
CORE_IDS = list(range(NCORES))

IN_NAMES = ("t_out", "t_ntg", "t_gt1", "t_gt2", "t_s1")
FP8_NAMES = ("t_out", "t_ntg", "t_gt1", "t_gt2")
MOV_NAMES = ("t_out", "t_gt1", "t_gt2")   # diff minuends

_CACHE = {}
LAST_RESULTS = None


def _job_tables(chunk, n_dve):
    """ACT/DVE split of the 3*n_tiles d^2 square-sum jobs (k, t, c):
    stat k in {0:d0, 1:d1, 2:d2}, batch t, chunk c.  Ln(s1) and s1^2 are
    always on ACT."""
    nchunk = F // chunk
    all_jobs = [(k, t, c) for t in range(BL) for c in range(nchunk)
                for k in range(3)]
    n_all = len(all_jobs)
    assert 0 <= n_dve <= n_all
    dve_idx = set()
    if n_dve:
        for i in range(n_dve):
            dve_idx.add(round(i * n_all / n_dve) % n_all)
        i = 0
        while len(dve_idx) < n_dve:
            if i not in dve_idx:
                dve_idx.add(i)
            i += 1
    dve_jobs = [j for i, j in enumerate(all_jobs) if i in dve_idx]
    act_jobs = [j for i, j in enumerate(all_jobs) if i not in dve_idx]
    return act_jobs, dve_jobs, nchunk


def _build(reps=1, chunk=CHUNK, n_dve=N_DVE_JOBS, io_bufs=IO_BUFS,
           mode="full"):
    # reps>1 wraps the streaming loop in a hardware For_i (same result each
    # repetition) — used only for repetition-delta timing.  The graded path
    # always builds with reps=1, mode="full".
    # mode: "full" | "dma" (loads only) | "compute" (load once, loop compute)
    fp32 = mybir.dt.float32
    bf16 = mybir.dt.bfloat16
    fp8 = mybir.dt.float8e4
    act_jobs, dve_jobs, nchunk = _job_tables(chunk, n_dve)
    nseg = chunk // SEG
    n_tiles = BL * nchunk
    n_bn = max(len(dve_jobs), 1)
    # acc columns: ACT d-jobs use nseg columns each (one per PSUM segment),
    # then one column per s1^2 job and one per Ln job.
    col_d = {j: i * nseg for i, j in enumerate(act_jobs)}
    col_s1sq0 = len(act_jobs) * nseg
    col_ln0 = col_s1sq0 + n_tiles
    n_cols = col_ln0 + n_tiles
    bn_slot = {j: i for i, j in enumerate(dve_jobs)}

    nc = bacc.Bacc(
        "TRN2", target_bir_lowering=False, debug=False, num_devices=NCORES
    )
    ins = {}
    for name in IN_NAMES:
        dt_in = fp8 if name in FP8_NAMES else bf16
        ins[name] = nc.dram_tensor(
            name, [BL, P, F], dt_in, kind="ExternalInput"
        ).ap()
    ident_in = nc.dram_tensor("ident", [P, P], fp8, kind="ExternalInput").ap()
    acc_out = nc.dram_tensor("acc", [P, n_cols], fp32,
                             kind="ExternalOutput").ap()
    bn_out = nc.dram_tensor(
        "bn", [P, n_bn, nseg, 6], fp32, kind="ExternalOutput"
    ).ap()

    Sq = mybir.ActivationFunctionType.Square
    Ln = mybir.ActivationFunctionType.Ln

    with tile.TileContext(nc) as tc:
        with (
            tc.tile_pool(name="io", bufs=io_bufs) as io_pool,
            tc.tile_pool(name="scr", bufs=1) as scr_pool,
            tc.tile_pool(name="acc", bufs=1) as acc_pool,
            tc.psum_pool(name="ps", bufs=2) as ps_pool,
        ):
            acc = acc_pool.tile([P, n_cols], fp32, tag="acc", name="acc")
            bn = acc_pool.tile([P, n_bn, nseg, 6], fp32, tag="bn", name="bn")
            ident = acc_pool.tile([P, P], fp8, tag="ident", name="ident")
            scr_act = scr_pool.tile([P, chunk], bf16, tag="scr_act",
                                    name="scr_act")
            nc.sync.dma_start(ident[:], ident_in)
            if mode == "dma" or not dve_jobs:
                nc.any.memset(bn[:], 0.0)

            def load(name, t, cs):
                dt_t = bf16 if name == "t_s1" else fp8
                tl = io_pool.tile([P, chunk], dt_t, tag=name, name=name)
                nc.sync.dma_start(tl[:], ins[name][t, :, cs])
                return tl

            fixed = {}
            if mode == "compute":
                for name in IN_NAMES:
                    fixed[name] = load(name, 0, slice(0, chunk))

            rep_loop = tc.For_i(0, reps, 1) if reps > 1 else nullcontext()
            with rep_loop:
                for t in range(BL):
                    for c in range(nchunk):
                        cs = slice(c * chunk, (c + 1) * chunk)
                        if mode == "compute":
                            tl = fixed
                        else:
                            tl = {name: load(name, t, cs)
                                  for name in IN_NAMES}
                        if mode == "dma":
                            nc.scalar.activation(
                                scr_act[:, 0:128], tl["t_s1"][:, 0:128], Sq,
                                accum_out=acc[:, 0:1],
                            )
                            continue

                        # s1 stats on ACT from SBUF bf16
                        i_t = t * nchunk + c
                        nc.scalar.activation(
                            scr_act[:], tl["t_s1"][:], Sq,
                            accum_out=acc[:, col_s1sq0 + i_t:
                                          col_s1sq0 + i_t + 1],
                        )
                        nc.scalar.activation(
                            scr_act[:], tl["t_s1"][:], Ln,
                            accum_out=acc[:, col_ln0 + i_t:
                                          col_ln0 + i_t + 1],
                        )

                        # d_k = I @ m_k + I @ (-target) per 512-col segment,
                        # then square+reduce from PSUM on ACT or DVE
                        for k, name in enumerate(MOV_NAMES):
                            j = (k, t, c)
                            for g in range(nseg):
                                gs = slice(g * SEG, (g + 1) * SEG)
                                ps = ps_pool.tile([P, SEG], fp32,
                                                  tag=f"psd{k}",
                                                  name=f"psd{k}")
                                nc.tensor.matmul(
                                    ps[:], ident[:], tl[name][:, gs],
                                    start=True, stop=False,
                                )
                                nc.tensor.matmul(
                                    ps[:], ident[:], tl["t_ntg"][:, gs],
                                    start=False, stop=True,
                                )
                                if j in bn_slot:
                                    nc.vector.bn_stats(
                                        bn[:, bn_slot[j], g, :], ps[:]
                                    )
                                else:
                                    col = col_d[j] + g
                                    nc.scalar.activation(
                                        scr_act[:, 0:SEG], ps[:], Sq,
                                        accum_out=acc[:, col: col + 1],
                                    )

            nc.sync.dma_start(acc_out, acc[:])
            nc.sync.dma_start(bn_out, bn[:])

    nc.compile()
    nc._job_meta = (chunk, act_jobs, dve_jobs, nchunk, nseg,
                    col_d, col_s1sq0, col_ln0)
    return nc


def _get_nc():
    if "nc" not in _CACHE:
        _CACHE["nc"] = _build()
    return _CACHE["nc"]


def _to_shards(name, arr):
    a = np.asarray(arr, dtype=np.float32)
    if name == "t_ntg":
        a = -a
    dt = (ml_dtypes.float8_e4m3fn if name in FP8_NAMES
          else ml_dtypes.bfloat16)
    a = a.astype(dt)
    return [np.ascontiguousarray(a[i * BL:(i + 1) * BL].reshape(BL, P, F))
            for i in CORE_IDS]


def _identity_fp8():
    return np.eye(P, dtype=np.float32).astype(ml_dtypes.float8_e4m3fn)


def _reduce_stats(results, job_meta):
    """Reconstruct S0..S2 per (core,batch,point), global sq_sum, ln_sum."""
    (chunk, act_jobs, dve_jobs, nchunk, nseg,
     col_d, col_s1sq0, col_ln0) = job_meta
    acc = np.stack([np.asarray(r["acc"], dtype=np.float64)
                    for r in results])          # [8, P, n_cols]
    bn = np.stack([np.asarray(r["bn"], dtype=np.float64)
                   for r in results])           # [8, P, n_bn, nseg, 6]

    S = np.zeros((3, NCORES, BL, P))            # stat k, core, batch, point
    for j, c0 in col_d.items():
        k, t, c = j
        S[k, :, t, :] += acc[:, :, c0:c0 + nseg].sum(axis=-1)
    for s, (k, t, c) in enumerate(dve_jobs):
        blk = bn[:, :, s, :, :]                 # [8, P, nseg, 6]
        sq = (blk[..., 2] + blk[..., 0] * blk[..., 1] ** 2
              + blk[..., 5] + blk[..., 3] * blk[..., 4] ** 2)
        S[k, :, t, :] += sq.sum(axis=-1)
    n_tiles = BL * nchunk
    sq_sum = acc[:, :, col_s1sq0:col_s1sq0 + n_tiles].sum()
    ln_sum = acc[:, :, col_ln0:col_ln0 + n_tiles].sum()
    return S, sq_sum, ln_sum


def kernel(out, target, gt0, gt1_mean, gt2_mean, gt2_var, leg, l_dynamic):
    global LAST_RESULTS
    nc = _get_nc()

    shards = {
        "t_out": _to_shards("t_out", out),
        "t_ntg": _to_shards("t_ntg", target),
        "t_gt1": _to_shards("t_gt1", gt1_mean),
        "t_gt2": _to_shards("t_gt2", gt2_mean),
        "t_s1": _to_shards("t_s1", gt2_var),
    }
    ident = _identity_fp8()
    in_maps = [
        {**{name: shards[name][i] for name in IN_NAMES}, "ident": ident}
        for i in CORE_IDS
    ]

    res = bass_utils.run_bass_kernel_spmd(nc, in_maps, CORE_IDS)
    LAST_RESULTS = res

    S, sq_sum, ln_sum = _reduce_stats(res.results, nc._job_meta)

    euc0 = np.sqrt(S[0]).sum() / 128.0
    euc1 = np.sqrt(S[1]).sum() / 128.0
    euc2 = np.sqrt(S[2]).sum() / 128.0
    s0, s1, s2 = S[0].sum(), S[1].sum(), S[2].sum()

    ntot = float(B * P * N * D)
    base_sum = ntot * np.log(2.0) - ln_sum + sq_sum / 8.0 - 0.5 * ntot
    kl = 1.4 * base_sum + (s0 + 0.2 * s1 + 0.2 * s2) / 8.0

    l_dyn, leg_v, gt0_v = float(l_dynamic), float(leg), float(gt0)
    outloss = euc0 + 0.01 * 0.2 * l_dyn * leg_v
    gt_loss = 0.1 * euc1 + 0.2 * euc2
    reg = gt0_v * 0.01 * l_dyn
    result = outloss + gt_loss + reg + kl / (1.2 * (euc0 + gt_loss))
    return np.asarray(result, dtype=np.float32)


# revision 21
# speedup vs baseline: 3.8832x; 1.1211x over previous
"""Trainium2 Bass kernel for the Deepeucloss loss function.

Computes a scalar loss from five [16, 128, 4096, 3] f32 tensors plus three
scalars.  Data-parallel across 8 NeuronCores: each core takes 2 of the 16
batches and streams its five input shards through SBUF once.

Math (NUM_CLASSES=128, L2_LAMBDA=0.01, S2=2.0):
  euc(m)   = sum_{b,p} sqrt(sum_{n,d} (m - target)^2) / 128
  base     = log(2/s1) + s1^2/8 - 0.5          (s1 = gt2_var)
  kl       = 1.4*sum(base) + (S0 + 0.2*S1 + 0.2*S2)/8,
             Sk = sum((m_k - target)^2)
  outloss  = euc(out) + 0.002*l_dynamic*leg
  gt_loss  = 0.1*euc(gt1_mean) + 0.2*euc(gt2_mean)
  reg      = gt0 * 0.01 * l_dynamic
  result   = outloss + gt_loss + reg + kl / (1.2*(euc(out) + gt_loss))

v2.6 design.  The f32 baseline sat at the ~358 GB/s per-core HBM roofline
(176 us).  Here the four N(0,1) tensors (out / negated-target / gt1 / gt2)
are stored fp8-e4m3 in HBM (quantization bias ~0.1%, far inside the 2e-2
gate) and gt2_var stays bf16, cutting per-core HBM bytes to 18.9 MB
(~53 us floor).  fp8 is slow on DVE (no 8-bit packing), so the diffs are
computed on the otherwise-idle PE tensor engine:

  d_k = I @ m_k + I @ (-target)     (PSUM fp32 accumulation, 512-col
                                     segments; target is negated on the
                                     host so the identity stationary
                                     never changes -> no weight reloads)

ACT (Square+accum_out) and DVE (bn_stats) then square-and-reduce the
per-(batch,point) sums directly from PSUM, split so both engines stay
under the DMA floor.  Ln(s1) and s1^2 run on ACT from the bf16 s1 tile
(Square is a filler function in the Ln table set -> no table thrash).

Device output per core: "acc" [128, n_cols] f32 activation accum columns
and "bn" [128, n_dve, chunk/512, 6] f32 bn_stats blocks
(sum(x^2) = count*var + count*mean^2).  The host reconstructs all sums in
float64 and finishes the scalar algebra.
"""

from contextlib import nullcontext

import numpy as np
import ml_dtypes

import concourse.bacc as bacc
import concourse.tile as tile
import concourse.mybir as mybir
from concourse import bass_utils

B, P, N, D = 16, 128, 4096, 3
F = N * D                      # 12288 elements per (batch, point) row
NCORES = 8
BL = B // NCORES               # batches per core
CHUNK = 4096
SEG = 512                      # PSUM bank free-dim capacity (fp32)
N_DVE_JOBS = 14
IO_BUFS = 3
CORE_IDS = list(range(NCORES))

IN_NAMES = ("t_out", "t_ntg", "t_gt1", "t_gt2", "t_s1")
FP8_NAMES = ("t_out", "t_ntg", "t_gt1", "t_gt2")
MOV_NAMES = ("t_out", "t_gt1", "t_gt2")   # diff minuends

_CACHE = {}
LAST_RESULTS = None


def _job_tables(chunk, n_dve):
    """ACT/DVE split of the 3*n_tiles d^2 square-sum jobs (k, t, c):
    stat k in {0:d0, 1:d1, 2:d2}, batch t, chunk c.  Ln(s1) and s1^2 are
    always on ACT."""
    nchunk = F // chunk
    all_jobs = [(k, t, c) for t in range(BL) for c in range(nchunk)
                for k in range(3)]
    n_all = len(all_jobs)
    assert 0 <= n_dve <= n_all
    dve_idx = set()
    if n_dve:
        for i in range(n_dve):
            dve_idx.add(round(i * n_all / n_dve) % n_all)
        i = 0
        while len(dve_idx) < n_dve:
            if i not in dve_idx:
                dve_idx.add(i)
            i += 1
    dve_jobs = [j for i, j in enumerate(all_jobs) if i in dve_idx]
    act_jobs = [j for i, j in enumerate(all_jobs) if i not in dve_idx]
    return act_jobs, dve_jobs, nchunk


def _build(reps=1, chunk=CHUNK, n_dve=N_DVE_JOBS, io_bufs=IO_BUFS,
           psw=2, ps_shared="mixed", ps_bufs=2, mode="full"):
    # reps>1 wraps the streaming loop in a hardware For_i (same result each
    # repetition) — used only for repetition-delta timing.  The graded path
    # always builds with reps=1, mode="full".
    # mode: "full" | "dma" (loads only) | "compute" (load once, loop compute)
    # psw: PSUM banks per tile ([P, psw*SEG]); ACT consumes a whole tile in
    #      one op, DVE bn_stats still reads per-512 segment.
    # ps_shared: one PSUM tag for all three diff stats (deeper rotation)
    #      instead of one tag per stat.
    fp32 = mybir.dt.float32
    bf16 = mybir.dt.bfloat16
    fp8 = mybir.dt.float8e4
    act_jobs, dve_jobs, nchunk = _job_tables(chunk, n_dve)
    nseg = chunk // SEG
    assert nseg % psw == 0
    ngrp = nseg // psw                       # PSUM tiles per (job, chunk)
    n_tiles = BL * nchunk
    n_bn = max(len(dve_jobs), 1)
    # acc columns: ACT d-jobs use ngrp columns each (one per PSUM tile),
    # then one column per s1^2 job and one per Ln job.
    col_d = {j: i * ngrp for i, j in enumerate(act_jobs)}
    col_s1sq0 = len(act_jobs) * ngrp
    col_ln0 = col_s1sq0 + n_tiles
    n_cols = col_ln0 + n_tiles
    bn_slot = {j: i for i, j in enumerate(dve_jobs)}

    nc = bacc.Bacc(
        "TRN2", target_bir_lowering=False, debug=False, num_devices=NCORES
    )
    ins = {}
    for name in IN_NAMES:
        dt_in = fp8 if name in FP8_NAMES else bf16
        ins[name] = nc.dram_tensor(
            name, [BL, P, F], dt_in, kind="ExternalInput"
        ).ap()
    ident_in = nc.dram_tensor("ident", [P, P], fp8, kind="ExternalInput").ap()
    acc_out = nc.dram_tensor("acc", [P, n_cols], fp32,
                             kind="ExternalOutput").ap()
    bn_out = nc.dram_tensor(
        "bn", [P, n_bn, nseg, 6], fp32, kind="ExternalOutput"
    ).ap()

    Sq = mybir.ActivationFunctionType.Square
    Ln = mybir.ActivationFunctionType.Ln

    with tile.TileContext(nc) as tc:
        with (
            tc.tile_pool(name="io", bufs=io_bufs) as io_pool,
            tc.tile_pool(name="scr", bufs=1) as scr_pool,
            tc.tile_pool(name="acc", bufs=1) as acc_pool,
            tc.psum_pool(name="ps", bufs=ps_bufs) as ps_pool,
        ):
            acc = acc_pool.tile([P, n_cols], fp32, tag="acc", name="acc")
            bn = acc_pool.tile([P, n_bn, nseg, 6], fp32, tag="bn", name="bn")
            ident = acc_pool.tile([P, P], fp8, tag="ident", name="ident")
            scr_act = scr_pool.tile([P, chunk], bf16, tag="scr_act",
                                    name="scr_act")
            nc.sync.dma_start(ident[:], ident_in)
            if mode == "dma" or not dve_jobs:
                nc.any.memset(bn[:], 0.0)

            def load(name, t, cs):
                dt_t = bf16 if name == "t_s1" else fp8
                tl = io_pool.tile([P, chunk], dt_t, tag=name, name=name)
                nc.sync.dma_start(tl[:], ins[name][t, :, cs])
                return tl

            fixed = {}
            if mode == "compute":
                for name in IN_NAMES:
                    fixed[name] = load(name, 0, slice(0, chunk))

            rep_loop = tc.For_i(0, reps, 1) if reps > 1 else nullcontext()
            with rep_loop:
                for t in range(BL):
                    for c in range(nchunk):
                        cs = slice(c * chunk, (c + 1) * chunk)
                        if mode == "compute":
                            tl = fixed
                        else:
                            tl = {name: load(name, t, cs)
                                  for name in IN_NAMES}
                        if mode == "dma":
                            nc.scalar.activation(
                                scr_act[:, 0:128], tl["t_s1"][:, 0:128], Sq,
                                accum_out=acc[:, 0:1],
                            )
                            continue

                        # s1 stats on ACT from SBUF bf16
                        i_t = t * nchunk + c
                        nc.scalar.activation(
                            scr_act[:], tl["t_s1"][:], Sq,
                            accum_out=acc[:, col_s1sq0 + i_t:
                                          col_s1sq0 + i_t + 1],
                        )
                        nc.scalar.activation(
                            scr_act[:], tl["t_s1"][:], Ln,
                            accum_out=acc[:, col_ln0 + i_t:
                                          col_ln0 + i_t + 1],
                        )

                        # d_k = I @ m_k + I @ (-target) per 512-col segment
                        # (psw segments share one PSUM tile), then
                        # square+reduce from PSUM on ACT or DVE
                        for k, name in enumerate(MOV_NAMES):
                            j = (k, t, c)
                            is_dve = j in bn_slot
                            if ps_shared == "mixed":
                                # ACT tiles are psw banks wide (cheaper big
                                # reads); DVE bn is capped at 512 free so
                                # its tiles stay one bank
                                tag = "psv" if is_dve else "psa"
                                j_psw = 1 if is_dve else psw
                                j_bufs = ps_bufs * psw if is_dve else ps_bufs
                            elif ps_shared == "consumer":
                                tag = "psv" if is_dve else "psa"
                                j_psw, j_bufs = psw, ps_bufs
                            elif ps_shared:
                                tag, j_psw, j_bufs = "psd", psw, ps_bufs
                            else:
                                tag, j_psw, j_bufs = f"psd{k}", psw, ps_bufs
                            j_ngrp = nseg // j_psw
                            for gr in range(j_ngrp):
                                ps = ps_pool.tile([P, j_psw * SEG], fp32,
                                                  tag=tag, name=tag,
                                                  bufs=j_bufs)
                                for s in range(j_psw):
                                    g = gr * j_psw + s
                                    gs = slice(g * SEG, (g + 1) * SEG)
                                    pss = ps[:, s * SEG:(s + 1) * SEG]
                                    nc.tensor.matmul(
                                        pss, ident[:], tl[name][:, gs],
                                        start=True, stop=False,
                                    )
                                    nc.tensor.matmul(
                                        pss, ident[:], tl["t_ntg"][:, gs],
                                        start=False, stop=True,
                                    )
                                if is_dve:
                                    psv = ps[:].rearrange(
                                        "p (s f) -> p s f", s=j_psw
                                    )
                                    for s in range(j_psw):
                                        g = gr * j_psw + s
                                        nc.vector.bn_stats(
                                            bn[:, bn_slot[j], g, :],
                                            psv[:, s, :]
                                        )
                                else:
                                    col = col_d[j] + gr
                                    nc.scalar.activation(
                                        scr_act[:, 0:j_psw * SEG], ps[:], Sq,
                                        accum_out=acc[:, col: col + 1],
                                    )

            nc.sync.dma_start(acc_out, acc[:])
            nc.sync.dma_start(bn_out, bn[:])

    nc.compile()
    nc._job_meta = (chunk, act_jobs, dve_jobs, nchunk, ngrp,
                    col_d, col_s1sq0, col_ln0)
    return nc


def _get_nc():
    if "nc" not in _CACHE:
        _CACHE["nc"] = _build()
    return _CACHE["nc"]


def _to_shards(name, arr):
    a = np.asarray(arr, dtype=np.float32)
    if name == "t_ntg":
        a = -a
    dt = (ml_dtypes.float8_e4m3fn if name in FP8_NAMES
          else ml_dtypes.bfloat16)
    a = a.astype(dt)
    return [np.ascontiguousarray(a[i * BL:(i + 1) * BL].reshape(BL, P, F))
            for i in CORE_IDS]


def _identity_fp8():
    return np.eye(P, dtype=np.float32).astype(ml_dtypes.float8_e4m3fn)


def _reduce_stats(results, job_meta):
    """Reconstruct S0..S2 per (core,batch,point), global sq_sum, ln_sum."""
    (chunk, act_jobs, dve_jobs, nchunk, ngrp,
     col_d, col_s1sq0, col_ln0) = job_meta
    acc = np.stack([np.asarray(r["acc"], dtype=np.float64)
                    for r in results])          # [8, P, n_cols]
    bn = np.stack([np.asarray(r["bn"], dtype=np.float64)
                   for r in results])           # [8, P, n_bn, nseg, 6]

    S = np.zeros((3, NCORES, BL, P))            # stat k, core, batch, point
    for j, c0 in col_d.items():
        k, t, c = j
        S[k, :, t, :] += acc[:, :, c0:c0 + ngrp].sum(axis=-1)
    for s, (k, t, c) in enumerate(dve_jobs):
        blk = bn[:, :, s, :, :]                 # [8, P, nseg, 6]
        sq = (blk[..., 2] + blk[..., 0] * blk[..., 1] ** 2
              + blk[..., 5] + blk[..., 3] * blk[..., 4] ** 2)
        S[k, :, t, :] += sq.sum(axis=-1)
    n_tiles = BL * nchunk
    sq_sum = acc[:, :, col_s1sq0:col_s1sq0 + n_tiles].sum()
    ln_sum = acc[:, :, col_ln0:col_ln0 + n_tiles].sum()
    return S, sq_sum, ln_sum


def kernel(out, target, gt0, gt1_mean, gt2_mean, gt2_var, leg, l_dynamic):
    global LAST_RESULTS
    nc = _get_nc()

    shards = {
        "t_out": _to_shards("t_out", out),
        "t_ntg": _to_shards("t_ntg", target),
        "t_gt1": _to_shards("t_gt1", gt1_mean),
        "t_gt2": _to_shards("t_gt2", gt2_mean),
        "t_s1": _to_shards("t_s1", gt2_var),
    }
    ident = _identity_fp8()
    in_maps = [
        {**{name: shards[name][i] for name in IN_NAMES}, "ident": ident}
        for i in CORE_IDS
    ]

    res = bass_utils.run_bass_kernel_spmd(nc, in_maps, CORE_IDS)
    LAST_RESULTS = res

    S, sq_sum, ln_sum = _reduce_stats(res.results, nc._job_meta)

    euc0 = np.sqrt(S[0]).sum() / 128.0
    euc1 = np.sqrt(S[1]).sum() / 128.0
    euc2 = np.sqrt(S[2]).sum() / 128.0
    s0, s1, s2 = S[0].sum(), S[1].sum(), S[2].sum()

    ntot = float(B * P * N * D)
    base_sum = ntot * np.log(2.0) - ln_sum + sq_sum / 8.0 - 0.5 * ntot
    kl = 1.4 * base_sum + (s0 + 0.2 * s1 + 0.2 * s2) / 8.0

    l_dyn, leg_v, gt0_v = float(l_dynamic), float(leg), float(gt0)
    outloss = euc0 + 0.01 * 0.2 * l_dyn * leg_v
    gt_loss = 0.1 * euc1 + 0.2 * euc2
    reg = gt0_v * 0.01 * l_dyn
    result = outloss + gt_loss + reg + kl / (1.2 * (euc0 + gt_loss))
    return np.asarray(result, dtype=np.float32)
